# revision 33
# baseline (speedup 1.0000x reference)
"""RNN-T Joint network kernel for Trainium2 (Bass/Tile), 8-core SPMD.

Problem: out[b,t,u,v] = tanh(enc[b,t,:] + pred[b,u,:]) @ W[v,:] + bias[v]
  B=4, T=256, U=64, D=640, V=4096  (fp32 in/out)

Sharding: data-parallel over (B,T). Core i handles b = i//2, t in
[(i%2)*128, (i%2)*128+128). Each core computes an [128*64, 4096] slice of
the output; no collectives needed.

Device kernel (per core), PE-bound at ~216 ns per 512-row fp16 matmul
(2560 matmuls ~ 553 us of stream; measured total ~578 us):
  - host pre-permutes operands so the contraction dim D sits on SBUF
    partitions AND every DMA line is contiguous DRAM (read-DMA packets
    are one per SBUF partition line, so line size sets DMA throughput):
    enc+pred pack into one [128p, KC*192] tensor (3840B lines), W packs
    partition-major [128p, NB*KC*512] (40KB lines, 16KB packets).
  - hT[d, (t,u)] = tanh(predT[d,u] + encT[d,t]) via scalar-engine
    activation with per-partition bias (one instr per (d-chunk, t)).
  - PE matmul per (m-chunk, psum-bank): psum[128m, 512v] +=
    hT[k][:,m].T @ w[k,n] over 5 k-chunks; operands in fp16 so the PE
    streams 1 row/cycle and LDWEIGHTS overlaps the previous matmul
    (f32r serializes the weight load: 272 ns vs 216 ns per matmul;
    512 is the ISA max moving-elements per matmul).
  - per-bank epilogue: DVE adds bias PSUM->SBUF (fp16), then a 128KB
    DMA out; n-outer/k-inner keeps drains and output DMA evenly spread
    and shrinks the kernel tail to one bank's worth of work. Output
    rides the wire as fp16 (halves output DMA) and is widened to fp32
    on the host.
  - bias is NOT DMA-broadcast (2MB): a 16KB row is replicated across
    partitions on-chip with rank-1 PE matmuls ones.T @ bias_row, which
    also warms the PE p-state before the main stream.
  - the weight stream owns the sync queue in first-consumption bank
    order (finer ranges early) with outputs FIFO'd behind it; enc/pred
    and the bias row ride the scalar queue concurrently, so the first
    tanh/matmul isn't gated on the 4MB weight preload.
"""

import os
import sys

import numpy as np

if "/root/.axon_site/_ro/trn_rl_repo" not in sys.path:
    sys.path.append("/root/.axon_site/_ro/trn_rl_repo")

import concourse.mybir as mybir  # noqa: E402
import concourse.tile as tile  # noqa: E402
from concourse import bacc  # noqa: E402
from concourse.bass_utils import run_bass_kernel_spmd  # noqa: E402

B, T, U, D, V = 4, 256, 64, 640, 4096
N_CORES = 8
T_PER_CORE = T // (N_CORES // B)  # 128
ROWS = T_PER_CORE * U  # 8192 rows per core
KC = D // 128  # 5 k-chunks
NB = V // 512  # 8 psum banks per row-chunk
M_CHUNKS = ROWS // 128  # 64  (each = 2 t values x 64 u)
T_PER_M = 128 // U  # 2

# matmul dtype mode: "fp16"/"bf16" (1 cyc/row, hidden weight loads),
# "f32r" (1 cyc/row but serialized ldweights), "f32" (exact, 4 cyc/row)
MM_MODE = os.environ.get("JOINT_MM_MODE", "fp16")


def build_nc(mode: str):
    nc = bacc.Bacc("TRN2", target_bir_lowering=False, debug=False)

    f32 = mybir.dt.float32
    w_dt = {
        "bf16": mybir.dt.bfloat16,
        "fp16": mybir.dt.float16,
        "f32r": mybir.dt.float32r,
    }.get(mode, f32)

    EP = T_PER_CORE + U  # 192 columns per k-chunk: enc t-values then pred u-values
    ep_d = nc.dram_tensor("encpredP", [128, KC * EP], f32, kind="ExternalInput")
    wP_d = nc.dram_tensor("wP", [128, NB * KC * 512], w_dt, kind="ExternalInput")
    # bias comes over the wire as one [1, V+128] fp32 row: bias then 128
    # ones (the rank-1 stationary for on-chip partition broadcast);
    # beats DMAing a 2MB broadcast into SBUF.
    bias_d = nc.dram_tensor("bias", [1, V + 128], mybir.dt.float32r, kind="ExternalInput")
    out_dt = w_dt if mode in ("fp16", "bf16") else f32
    # bank-major output layout: each per-bank [128-row, 512] store is a
    # fully contiguous 128KB DRAM block (16KB write packets instead of
    # 1KB row-runs); the host reassembles [NB, ROWS, 512] -> [ROWS, V].
    out_d = nc.dram_tensor("out", [NB * ROWS, 512], out_dt, kind="ExternalOutput")

    epP = ep_d.ap().rearrange("p (k e) -> p k e", k=KC)
    wP = wP_d.ap().rearrange("p (n k c) -> p n k c", n=NB, k=KC)
    out = out_d.ap().rearrange("(n r) c -> n r c", n=NB)

    h_dt = w_dt

    with tile.TileContext(nc) as tc:
        with (
            tc.tile_pool(name="singles", bufs=1) as singles,
            tc.tile_pool(name="hpool", bufs=4) as hpool,
            tc.tile_pool(name="opool", bufs=20) as opool,
            tc.tile_pool(name="psum", bufs=8, space="PSUM") as psum_pool,
        ):
            # enc+pred ride one packed DMA (3840B lines, ~0.5MB) on the
            # scalar queue, concurrent with the weight stream on the sync
            # queue — they gate the very first tanh.
            ep_s = singles.tile([128, KC, EP], f32, tag="ep")
            nc.scalar.dma_start(out=ep_s, in_=epP)
            # weights: DRAM is partition-major (40KB contiguous/partition in
            # bank-then-k order) so read packets aggregate well (read-DMA
            # packets are one per SBUF partition line). Column-range DMAs
            # stage banks in consumption order, finer early.
            w_all = singles.tile([128, NB, KC, 512], w_dt, tag="w")
            for lo, hi in ((0, 1), (1, 2), (2, 3), (3, 5), (5, 8)):
                nc.sync.dma_start(
                    out=w_all[:, lo:hi], in_=wP[:, lo:hi]
                )
            # bias: 16KB row DMA on the scalar queue (hoisted early, no
            # contention), then replicate across partitions with rank-1
            # PE matmuls ones.T @ bias_row — this also warms the PE
            # p-state before the main stream begins.
            br_s = singles.tile([1, V + 128], mybir.dt.float32r, tag="br")
            nc.scalar.dma_start(out=br_s, in_=bias_d.ap())
            bias_s = singles.tile([128, V], f32, tag="bias")
            for n in range(NB):
                pb = psum_pool.tile([128, 512], mybir.dt.float32, tag="ps", name="ps")
                nc.tensor.matmul(
                    pb,
                    br_s[0:1, V : V + 128],
                    br_s[0:1, n * 512 : (n + 1) * 512],
                    start=True,
                    stop=True,
                )
                nc.vector.tensor_copy(bias_s[:, n * 512 : (n + 1) * 512], pb)

            for m in range(M_CHUNKS):
                hT = hpool.tile([128, KC, 128], h_dt, tag="hT")
                for k in range(KC):
                    for j in range(T_PER_M):
                        t = m * T_PER_M + j
                        nc.scalar.activation(
                            out=hT[:, k, j * U : (j + 1) * U],
                            in_=ep_s[:, k, T_PER_CORE:],
                            func=mybir.ActivationFunctionType.Tanh,
                            bias=ep_s[:, k, t : t + 1],
                        )

                for n in range(NB):
                    ps = psum_pool.tile([128, 512], mybir.dt.float32, tag="ps", name="ps")
                    for k in range(KC):
                        nc.tensor.matmul(
                            ps,
                            hT[:, k, :],
                            w_all[:, n, k, :],
                            start=(k == 0),
                            stop=(k == KC - 1),
                        )
                    ob = opool.tile([128, 512], out_dt, tag="ob")
                    nc.vector.tensor_add(
                        ob, ps, bias_s[:, n * 512 : (n + 1) * 512]
                    )
                    nc.sync.dma_start(
                        out=out[n, m * 128 : (m + 1) * 128, :],
                        in_=ob,
                    )

    nc.compile()
    return nc


_NC_CACHE = {}


def _get_nc(mode: str):
    if mode not in _NC_CACHE:
        _NC_CACHE[mode] = build_nc(mode)
    return _NC_CACHE[mode]


def _pack_ep(enc, pred):
    """enc [T_PER_CORE, D], pred [U, D] -> [128p, KC*(T_PER_CORE+U)]
    with per-k layout [enc t-values | pred u-values], row d = k*128 + p."""
    e = enc.T.reshape(KC, 128, T_PER_CORE)
    p = pred.T.reshape(KC, 128, U)
    return np.ascontiguousarray(
        np.concatenate([e, p], axis=2).transpose(1, 0, 2).reshape(128, -1)
    )


def kernel(enc_out, pred_out, W_out, b_out, _trace=False):
    if not _trace:
        # the axon trace path needs antenv.axon_hooks, absent here
        os.environ["BASS_NEVER_TRACE"] = "1"
    enc_out = np.asarray(enc_out, dtype=np.float32)
    pred_out = np.asarray(pred_out, dtype=np.float32)
    W_out = np.asarray(W_out, dtype=np.float32)
    b_out = np.asarray(b_out, dtype=np.float32)

    mode = MM_MODE
    nc = _get_nc(mode)

    wT = W_out.T  # [D, V]
    if mode == "bf16":
        import ml_dtypes

        wT = wT.astype(ml_dtypes.bfloat16)
    elif mode == "fp16":
        wT = wT.astype(np.float16)
    # partition-major pack: wP[p, n, k, c] = wT[k*128+p, n*512+c]
    wP = np.ascontiguousarray(
        wT.reshape(KC, 128, NB, 512).transpose(1, 2, 0, 3).reshape(128, NB * KC * 512)
    )
    bias2d = np.ascontiguousarray(
        np.concatenate([b_out, np.ones(128, np.float32)]).reshape(1, V + 128)
    )

    in_maps = []
    for i in range(N_CORES):
        b_idx = i // (N_CORES // B)
        t0 = (i % (N_CORES // B)) * T_PER_CORE
        in_maps.append(
            {
                "encpredP": _pack_ep(
                    enc_out[b_idx, t0 : t0 + T_PER_CORE], pred_out[b_idx]
                ),
                "wP": wP,
                "bias": bias2d,
            }
        )

    res = run_bass_kernel_spmd(
        nc, in_maps, core_ids=list(range(N_CORES)), trace=_trace
    )

    out = np.empty((B, T, U, V), dtype=np.float32)
    for i in range(N_CORES):
        b_idx = i // (N_CORES // B)
        t0 = (i % (N_CORES // B)) * T_PER_CORE
        ri = res.results[i]["out"]
        out[b_idx, t0 : t0 + T_PER_CORE] = (
            np.asarray(ri)
            .reshape(NB, T_PER_CORE, U, 512)
            .transpose(1, 2, 0, 3)
            .reshape(T_PER_CORE, U, V)
        )
    if _trace:
        return out, res
    return out
